# revision 1
# baseline (speedup 1.0000x reference)
"""Trainium2 Bass kernel for nn_MultiHeadAttention_67379446939752.

Per-token multi-head attention:
  Q = q @ Wq.T + bq ; K,V likewise        [B,S,D] -> [B,S,H,HD]
  score[t,h,g] = sum_d Q[t,h,d] K[t,g,d]  (per-token HxH gram, no seq mixing)
  attn[t] = softmax(score[t]) @ V[t]      -> [B,S,D]
  out = attn @ Wo.T + bo

Strategy: data-parallel over the 16384 tokens across 8 NeuronCores (2048
tokens/core).  All big matmuls run in float32r (full PE rate at N>=256,
~1e-4 relative error).  Host pre-transposes activations/weights so the
contraction dim lands on SBUF partitions with no on-device transposes.
The per-token 16x16 attention is computed 8 tokens at a time as a single
128x128x128 fp32 matmul whose cross-token blocks are pushed to -1024 in
PSUM by a rank-8 bf16 mask matmul; exp() then zeroes them exactly, so the
block-diagonal softmax needs no masking pass on DVE.
"""
import sys
sys.path.insert(0, "/opt/trn_rl_repo")
import numpy as np
import concourse.bass as bass
import concourse.mybir as mybir
import concourse.bacc as bacc
import concourse.tile as tile
from concourse.bass_utils import run_bass_kernel_spmd

B, S, D, H, HD = 4, 4096, 2048, 16, 128
NCORES = 8
T_FULL = B * S
F32, F32R, BF16 = mybir.dt.float32, mybir.dt.float32r, mybir.dt.bfloat16
KT = D // 128            # contraction tiles
SHIFT = 25.0             # constant softmax shift (softmax-invariant)
NEG = 1024.0             # additive mask magnitude for cross-token blocks
TA = 256                 # token chunk (phase A/B/C share this granularity)
Exp = mybir.ActivationFunctionType.Exp


def mask_consts():
    # u8[r,(t,h)] = 1 if t==r ; v8[r,(t',g)] = -NEG*(1 - (t'==r))
    u = np.zeros((8, 128), np.float32)
    for r in range(8):
        u[r, r * 16:(r + 1) * 16] = 1.0
    v = np.full((8, 128), -NEG, np.float32)
    for r in range(8):
        v[r, r * 16:(r + 1) * 16] = 0.0
    return u, v


def build(T, debug=False, repeat=1, trace_sim=False):
    import ml_dtypes
    TAe = min(TA, T)
    NCH = T // TAe           # chunks
    NBK = TAe // 8           # 8-token blocks per chunk
    nc = bacc.Bacc(None, target_bir_lowering=False)
    dt_in = lambda n, s: nc.dram_tensor(n, s, F32R, kind="ExternalInput")
    qT = dt_in("qT", [D, T]); kT = dt_in("kT", [D, T]); vT = dt_in("vT", [D, T])
    WqT = dt_in("WqT", [D, D]); WkT = dt_in("WkT", [D, D])
    WvT = dt_in("WvT", [D, D]); WoT = dt_in("WoT", [D, D])
    bqT = nc.dram_tensor("bqT", [128, H], F32, kind="ExternalInput")
    bkT = nc.dram_tensor("bkT", [128, H], F32, kind="ExternalInput")
    bvT = nc.dram_tensor("bvT", [128, H], F32, kind="ExternalInput")
    bo_row = nc.dram_tensor("bo_row", [1, D], F32R, kind="ExternalInput")
    ones_row = nc.dram_tensor("ones_row", [1, 128], F32R, kind="ExternalInput")
    out_d = nc.dram_tensor("out", [T, D], F32, kind="ExternalOutput")
    dbg = {}
    if debug:
        for n, shp in (("dQT", [128, T * H]), ("dKT", [128, T * H]),
                       ("dV", [128, T * H]), ("dATT", [D, T])):
            dbg[n] = nc.dram_tensor(n, shp, F32, kind="ExternalOutput")

    u8_np, v8_np = mask_consts()
    u8_d = nc.inline_tensor(u8_np.astype(ml_dtypes.bfloat16), "u8c")
    v8_d = nc.inline_tensor(v8_np.astype(ml_dtypes.bfloat16), "v8c")
    id_d = nc.inline_tensor(np.eye(128, dtype=np.float32), "id128").bitcast(F32R)

    with tile.TileContext(nc, trace_sim=trace_sim) as tc:
        with (
            tc.tile_pool(name="dram", bufs=1, space="DRAM") as dpool,
            tc.tile_pool(name="const", bufs=1) as cpool,
        ):
            # per-chunk spill tiles (fine-grained cross-phase deps)
            QT_ds = [dpool.tile([128, TAe * H], F32R, tag=f"QTd{i}", name=f"QTd{i}") for i in range(NCH)]
            KT_ds = [dpool.tile([128, TAe * H], F32R, tag=f"KTd{i}", name=f"KTd{i}") for i in range(NCH)]
            VT_ds = [dpool.tile([128, TAe * H], F32R, tag=f"VTd{i}", name=f"VTd{i}") for i in range(NCH)]
            ATT_ds = [dpool.tile([D, TAe], F32R, tag=f"ATTd{i}", name=f"ATTd{i}") for i in range(NCH)]

            u8 = cpool.tile([8, 128], BF16, tag="u8")
            v8 = cpool.tile([8, 128], BF16, tag="v8")
            ident = cpool.tile([128, 128], F32R, tag="ident")
            nc.sync.dma_start(u8[:], u8_d[:])
            nc.sync.dma_start(v8[:], v8_d[:])
            nc.sync.dma_start(ident[:], id_d[:])
            biasq = cpool.tile([128, H], F32, tag="bq")
            biask = cpool.tile([128, H], F32, tag="bk")
            biasv = cpool.tile([128, H], F32, tag="bvt")
            bor = cpool.tile([1, D], F32R, tag="bo")
            onesr = cpool.tile([1, 128], F32R, tag="ones")
            nc.sync.dma_start(biasq[:], bqT[:])
            nc.sync.dma_start(biask[:], bkT[:])
            nc.sync.dma_start(biasv[:], bvT[:])
            nc.sync.dma_start(bor[:], bo_row[:])
            nc.sync.dma_start(onesr[:], ones_row[:])
            shiftc = cpool.tile([128, 1], F32, tag="shiftc")
            nc.vector.memset(shiftc[:], -SHIFT)

            def _load_w(pool, win, tag):
                # separate quarter tiles -> first matmuls start after 1/4 load
                src = win.ap().rearrange("(it p) j -> p it j", p=128)
                parts = []
                for q in range(4):
                    wq = pool.tile([128, 4, D], F32R, tag=f"{tag}{q}", name=f"{tag}{q}")
                    nc.sync.dma_start(wq[:], src[:, q * 4:(q + 1) * 4, :])
                    parts.append(wq)
                return parts

            def _phases():
                # ---------------- Phase A: projections ----------------
                with (
                    tc.tile_pool(name="wt", bufs=1) as wpool,
                    tc.tile_pool(name="xs", bufs=2) as xpool,
                    tc.tile_pool(name="psA", bufs=8, space="PSUM") as psA,
                    tc.tile_pool(name="stA", bufs=1) as stA,
                ):
                    for xin, win, bias, spills in (
                        (qT, WqT, biasq, QT_ds),
                        (kT, WkT, biask, KT_ds),
                        (vT, WvT, biasv, VT_ds),
                    ):
                        xs0 = xpool.tile([128, KT, TAe], F32R, tag="xs", name="xs0")
                        nc.sync.dma_start(
                            xs0[:], xin[:, 0:TAe].rearrange("(it p) t -> p it t", p=128))
                        wt = _load_w(wpool, win, "wt")
                        for c in range(NCH):
                            if c == 0:
                                xs = xs0
                            else:
                                xs = xpool.tile([128, KT, TAe], F32R, tag="xs")
                                nc.sync.dma_start(
                                    xs[:], xin[:, c * TAe:(c + 1) * TAe].rearrange(
                                        "(it p) t -> p it t", p=128))
                            stg = stA.tile([128, TAe, H], F32R, tag="stA")
                            for jh in range(2):
                                pss = [psA.tile([128, TAe], F32, tag="psA",
                                                name=f"psA{jh}_{j}") for j in range(8)]
                                for q in range(4):
                                    for jl in range(8):
                                        jt = jh * 8 + jl
                                        for kl in range(4):
                                            ki = q * 4 + kl
                                            nc.tensor.matmul(
                                                pss[jl][:], wt[q][:, kl, jt * 128:(jt + 1) * 128],
                                                xs[:, ki, :], start=(ki == 0), stop=(ki == KT - 1))
                                for jl in range(8):
                                    jt = jh * 8 + jl
                                    nc.any.tensor_scalar_add(stg[:, :, jt], pss[jl][:],
                                                             bias[:, jt:jt + 1])
                            nc.sync.dma_start(
                                spills[c][:], stg[:].rearrange("p t h -> p (t h)"))

                # ---------------- Phase B (with Wo q0 prefetch) + C ----------------
                with tc.tile_pool(name="wo", bufs=1) as wopool:
                  wsrc = WoT.ap().rearrange("(h p) j -> p h j", p=128)
                  wo0 = wopool.tile([128, 4, D], F32R, tag="wo0", name="wo0")
                  nc.sync.dma_start(wo0[:], wsrc[:, 0:4, :])
                  with (
                      tc.tile_pool(name="qk", bufs=2) as qkpool,
                      tc.tile_pool(name="vb", bufs=2) as vpool,
                      tc.tile_pool(name="attc", bufs=2) as apool,
                      tc.tile_pool(name="eb", bufs=6) as epool,
                      tc.tile_pool(name="zb", bufs=8) as zpool,
                      tc.tile_pool(name="psS", bufs=2, space="PSUM") as psS,
                      tc.tile_pool(name="psT", bufs=2, space="PSUM") as psT,
                      tc.tile_pool(name="psV", bufs=2, space="PSUM") as psV,
                      tc.tile_pool(name="psA2", bufs=2, space="PSUM") as psA2,
                  ):
                      for c in range(NCH):
                          t0c = c * TAe
                          QTs = qkpool.tile([128, TAe, H], F32R, tag="QTs")
                          KTs = qkpool.tile([128, TAe, H], F32R, tag="KTs")
                          nc.gpsimd.dma_start(
                              QTs[:], QT_ds[c][:].rearrange("p (t h) -> p t h", h=H))
                          nc.gpsimd.dma_start(
                              KTs[:], KT_ds[c][:].rearrange("p (t h) -> p t h", h=H))
                          VTs = vpool.tile([128, TAe, H], F32R, tag="VTs")
                          nc.sync.dma_start(
                              VTs[:], VT_ds[c][:].rearrange("p (t h) -> p t h", h=H))
                          ATTc = apool.tile([128, H, TAe], F32R, tag="ATTc")
                          for bk in range(NBK):
                              sl = slice(bk * 8, (bk + 1) * 8)
                              w0 = (bk // 2) * 2            # even-aligned 2-block window
                              off = (bk % 2) * 128          # valid column offset
                              slw = slice(w0 * 8, (w0 + 2) * 8)
                              ps_b = psS.tile([128, 256], F32, tag="ps_s")
                              nc.tensor.matmul(
                                  ps_b[:],
                                  QTs[:, sl, :].rearrange("p t h -> p (t h)"),
                                  KTs[:, slw, :].rearrange("p t h -> p (t h)"),
                                  start=True, stop=False, skip_group_check=True)
                              nc.tensor.matmul(ps_b[:, off:off + 128], u8[:], v8[:],
                                               start=False, stop=True, skip_group_check=True)
                              E = epool.tile([128, 128], F32, tag="E")
                              Z = zpool.tile([128, 1], F32, tag="Z")
                              nc.scalar.activation(E[:], ps_b[:, off:off + 128], Exp,
                                                   bias=shiftc[:], accum_out=Z[:])
                              R = zpool.tile([128, 1], F32, tag="R")
                              nc.vector.reciprocal(R[:], Z[:])
                              Wb = epool.tile([128, 128], F32R, tag="Wb")
                              nc.vector.tensor_scalar_mul(Wb[:], E[:], R[:])
                              ps_t = psT.tile([128, 128], F32R, tag="ps_t")
                              nc.tensor.transpose(ps_t[:], Wb[:], ident[:])
                              WTs = epool.tile([128, 128], F32R, tag="WTs")
                              nc.any.tensor_copy(WTs[:], ps_t[:])
                              ps_v = psV.tile([128, 128], F32R, tag="ps_v")
                              nc.tensor.transpose(
                                  ps_v[:], VTs[:, sl, :].rearrange("p t h -> p (t h)"),
                                  ident[:])
                              Vb = epool.tile([128, 128], F32R, tag="Vb")
                              nc.any.tensor_copy(Vb[:], ps_v[:])
                              ps_a = psA2.tile([128, 128], F32, tag="ps_a")
                              nc.tensor.matmul(ps_a[:], Vb[:], WTs[:],
                                               start=True, stop=True)
                              nc.any.tensor_copy(
                                  ATTc[:, :, bk * 8:(bk + 1) * 8].rearrange("p h t -> p t h"),
                                  ps_a[:].rearrange("p (t h) -> p t h", t=8))
                          nc.sync.dma_start(
                              ATT_ds[c][:].rearrange("(h p) t -> p h t", p=128), ATTc[:])

                  # ---------------- Phase C: output projection ----------------
                  with (
                      tc.tile_pool(name="wo", bufs=1) as wopool,
                      tc.tile_pool(name="ca", bufs=2) as capool,
                      tc.tile_pool(name="psC", bufs=8, space="PSUM") as psC,
                      tc.tile_pool(name="stC", bufs=4) as stC,
                  ):
                      ATTs0 = capool.tile([128, H, TAe], F32R, tag="ATTs", name="ATTs0")
                      nc.sync.dma_start(
                          ATTs0[:], ATT_ds[0][:].rearrange("(h p) t -> p h t", p=128))
                      wo = [wo0]
                      for q in range(1, 4):
                          wq = wopool.tile([128, 4, D], F32R, tag=f"wo{q}", name=f"wo{q}")
                          nc.sync.dma_start(wq[:], wsrc[:, q * 4:(q + 1) * 4, :])
                          wo.append(wq)
                      for cc in range(NCH):
                          if cc == 0:
                              ATTs = ATTs0
                          else:
                              ATTs = capool.tile([128, H, TAe], F32R, tag="ATTs")
                              nc.sync.dma_start(
                                  ATTs[:], ATT_ds[cc][:].rearrange("(h p) t -> p h t", p=128))
                          tjs = [(tt, jc) for tt in range(TAe // 128) for jc in range(D // 512)]
                          pss = [psC.tile([128, 512], F32, tag="psC", name=f"psC{i}")
                                 for i in range(len(tjs))]
                          for hq in range(4):
                              for i, (tt, jc) in enumerate(tjs):
                                  for hl in range(4):
                                      h = hq * 4 + hl
                                      nc.tensor.matmul(
                                          pss[i][:], ATTs[:, h, tt * 128:(tt + 1) * 128],
                                          wo[hq][:, hl, jc * 512:(jc + 1) * 512],
                                          start=(h == 0), stop=False)
                          for i, (tt, jc) in enumerate(tjs):
                              nc.tensor.matmul(pss[i][:], onesr[:], bor[:, jc * 512:(jc + 1) * 512],
                                               start=False, stop=True)
                              st = stC.tile([128, 512], F32, tag="stC")
                              nc.any.tensor_copy(st[:], pss[i][:])
                              nc.sync.dma_start(
                                  out_d[cc * TAe + tt * 128: cc * TAe + (tt + 1) * 128,
                                        jc * 512:(jc + 1) * 512], st[:])

            for _rep in range(repeat):
                _phases()

            if debug:
                with tc.tile_pool(name="dbgp", bufs=2) as dbgp:
                    def dump(name, srcs, width):
                        for i, srct in enumerate(srcs):
                            flat = srct[:].rearrange("p a b -> p (a b)") \
                                if len(srct.shape) == 3 else srct[:]
                            rows = flat.shape[0]
                            for r0 in range(0, rows, 128):
                                tcp = dbgp.tile([128, width], F32, tag="dbg")
                                nc.sync.dma_start(tcp[:], flat[r0:r0 + 128, :].bitcast(F32))
                                nc.sync.dma_start(
                                    dbg[name][r0:r0 + 128, i * width:(i + 1) * width], tcp[:])
                    dump("dQT", QT_ds, TAe * H)
                    dump("dKT", KT_ds, TAe * H)
                    dump("dV", VT_ds, TAe * H)
                    dump("dATT", ATT_ds, TAe)
    nc.compile()
    return nc


_cache = {}


def get_nc(T):
    if T not in _cache:
        _cache[T] = build(T)
    return _cache[T]


def make_in_maps(q, k, v, Wq, bq, Wk, bk, Wv, bv, Wo, bo, ncores=NCORES, T=None):
    f = np.float32
    q = np.asarray(q, f).reshape(-1, D)
    k = np.asarray(k, f).reshape(-1, D)
    v = np.asarray(v, f).reshape(-1, D)
    if T is None:
        T = q.shape[0] // ncores
    WqT = np.ascontiguousarray(np.asarray(Wq, f).T)
    WkT = np.ascontiguousarray(np.asarray(Wk, f).T)
    WvT = np.ascontiguousarray(np.asarray(Wv, f).T)
    WoT = np.ascontiguousarray(np.asarray(Wo, f).T)
    bqT = np.ascontiguousarray(np.asarray(bq, f).reshape(H, 128).T)
    bkT = np.ascontiguousarray(np.asarray(bk, f).reshape(H, 128).T)
    bvTc = np.ascontiguousarray(np.asarray(bv, f).reshape(H, 128).T)
    bor = np.asarray(bo, f).reshape(1, D)
    maps = []
    for c in range(ncores):
        sl = slice(c * T, (c + 1) * T)
        maps.append({
            "qT": np.ascontiguousarray(q[sl].T),
            "kT": np.ascontiguousarray(k[sl].T),
            "vT": np.ascontiguousarray(v[sl].T),
            "WqT": WqT, "WkT": WkT, "WvT": WvT, "WoT": WoT,
            "bqT": bqT, "bkT": bkT, "bvT": bvTc, "bo_row": bor,
            "ones_row": np.ones((1, 128), f),
        })
    return maps, T


def kernel(q, k, v, Wq, bq, Wk, bk, Wv, bv, Wo, bo):
    maps, T = make_in_maps(q, k, v, Wq, bq, Wk, bk, Wv, bv, Wo, bo)
    nc = get_nc(T)
    res = run_bass_kernel_spmd(nc, maps, list(range(NCORES)))
    out = np.concatenate([np.asarray(r["out"]) for r in res.results], axis=0)
    return out.reshape(B, S, D).astype(np.float32)



# revision 6
# speedup vs baseline: 2.8328x; 2.8328x over previous
"""Trainium2 Bass kernel for nn_MultiHeadAttention_67379446939752.

Per-token multi-head attention:
  Q = q @ Wq.T + bq ; K,V likewise        [B,S,D] -> [B,S,H,HD]
  score[t,h,g] = sum_d Q[t,h,d] K[t,g,d]  (per-token HxH gram, no seq mixing)
  attn[t] = softmax(score[t]) @ V[t]      -> [B,S,D]
  out = attn @ Wo.T + bo

v2 strategy (wall-clock per call is dominated by host<->device transfer, so
minimize wire bytes first, then keep HW exec near the PE roofline):
  - Data-parallel over the 16384 tokens across 8 NeuronCores (2048/core).
  - fp16 wire format for activations, weights and output (max rel err vs
    fp64 reference ~2.4e-3, an 8x margin under the 2e-2 gate).
  - Weights are sharded 8-ways on the wire (256 rows each) and AllGathered
    on-device over NeuronLink: 536MB of replicated weight traffic -> 33.5MB.
  - Natural [T,D]/[D,D] row-major layouts on the wire; the contraction-dim
    transposes happen on-device as cheap PE transpose ops (fp16: 128 cyc per
    128x128 tile), so the host does no big transposes.
  - All big matmuls in fp16 operands (full PE rate, fp32 PSUM accumulate).
  - The per-token 16x16 attention runs 8 tokens at a time as 128x128 fp16
    matmuls whose cross-token blocks are pushed to -1024 in PSUM by a rank-8
    mask matmul; exp() then zeroes them exactly (block-diagonal softmax with
    no DVE masking pass).
  - Attention and the output projection are fused per 256-token chunk (the
    attn result never round-trips through DRAM).
"""
import sys
sys.path.insert(0, "/opt/trn_rl_repo")
import numpy as np
import concourse.bass as bass
import concourse.mybir as mybir
import concourse.bacc as bacc
import concourse.tile as tile
from concourse.bass_utils import run_bass_kernel_spmd

B, S, D, H, HD = 4, 4096, 2048, 16, 128
NCORES = 8
T_FULL = B * S
F16, F32 = mybir.dt.float16, mybir.dt.float32
KT = D // 128            # contraction tiles
SHIFT = 25.0             # constant softmax shift (softmax-invariant)
NEG = 1024.0             # additive mask magnitude for cross-token blocks
TA = 256                 # token chunk
Exp = mybir.ActivationFunctionType.Exp


def mask_consts():
    # u8[r,(t,h)] = 1 if t==r ; v8[r,(t',g)] = -NEG*(1 - (t'==r))
    u = np.zeros((8, 128), np.float16)
    for r in range(8):
        u[r, r * 16:(r + 1) * 16] = 1.0
    v = np.full((8, 128), -NEG, np.float16)
    for r in range(8):
        v[r, r * 16:(r + 1) * 16] = 0.0
    return u, v


def build(T, ncores=NCORES, shared_gather=True):
    NCH = T // TA            # chunks
    NBK = TA // 8            # 8-token blocks per chunk
    NTB = TA // 128          # 128-token row tiles per chunk
    DS_ = D // ncores        # weight shard rows
    nc = bacc.Bacc(None, target_bir_lowering=False, num_devices=ncores)
    xq = nc.dram_tensor("xq", [T, D], F16, kind="ExternalInput")
    xk = nc.dram_tensor("xk", [T, D], F16, kind="ExternalInput")
    xv = nc.dram_tensor("xv", [T, D], F16, kind="ExternalInput")
    wqs = nc.dram_tensor("wqs", [DS_, D], F16, kind="ExternalInput")
    wks = nc.dram_tensor("wks", [DS_, D], F16, kind="ExternalInput")
    wvs = nc.dram_tensor("wvs", [DS_, D], F16, kind="ExternalInput")
    wos = nc.dram_tensor("wos", [DS_, D], F16, kind="ExternalInput")
    bq2 = nc.dram_tensor("bq2", [128, H], F32, kind="ExternalInput")
    bk2 = nc.dram_tensor("bk2", [128, H], F32, kind="ExternalInput")
    bv2 = nc.dram_tensor("bv2", [128, H], F32, kind="ExternalInput")
    bo_row = nc.dram_tensor("bo_row", [1, D], F16, kind="ExternalInput")
    ones_row = nc.dram_tensor("ones_row", [1, 128], F16, kind="ExternalInput")
    out_d = nc.dram_tensor("out", [T, D], F16, kind="ExternalOutput")

    u8_np, v8_np = mask_consts()
    u8_d = nc.inline_tensor(u8_np, "u8c")
    v8_d = nc.inline_tensor(v8_np, "v8c")
    id_d = nc.inline_tensor(np.eye(128, dtype=np.float16), "id128")

    with tile.TileContext(nc) as tc:
        with (
            tc.tile_pool(name="dram", bufs=1, space="DRAM") as dpool,
            tc.tile_pool(name="const", bufs=1) as cpool,
        ):
            # ---- on-device weight AllGather (shard -> full, fp16) ----
            Wg = []
            for i, wsh in enumerate((wqs, wks, wvs, wos)):
                gg = dpool.tile([D, D], F16, tag=f"Wg{i}", name=f"Wg{i}",
                                addr_space="Shared" if shared_gather else "Local")
                if ncores == 1:
                    nc.gpsimd.dma_start(gg[:], wsh[:])
                else:
                    bn = dpool.tile([DS_, D], F16, tag=f"bnc{i}", name=f"bnc{i}")
                    nc.gpsimd.dma_start(bn[:], wsh[:])
                    nc.gpsimd.collective_compute(
                        "AllGather", mybir.AluOpType.bypass,
                        replica_groups=[list(range(ncores))],
                        ins=[bn[:]], outs=[gg[:]])
                Wg.append(gg)

            # per-chunk QKV spill tiles (fine-grained cross-phase deps)
            QT_ds = [dpool.tile([128, TA * H], F16, tag=f"QTd{i}", name=f"QTd{i}") for i in range(NCH)]
            KT_ds = [dpool.tile([128, TA * H], F16, tag=f"KTd{i}", name=f"KTd{i}") for i in range(NCH)]
            VT_ds = [dpool.tile([128, TA * H], F16, tag=f"VTd{i}", name=f"VTd{i}") for i in range(NCH)]

            u8 = cpool.tile([8, 128], F16, tag="u8")
            v8 = cpool.tile([8, 128], F16, tag="v8")
            identF = cpool.tile([128, 128], F16, tag="identF")
            nc.sync.dma_start(u8[:], u8_d[:])
            nc.sync.dma_start(v8[:], v8_d[:])
            nc.sync.dma_start(identF[:], id_d[:])
            biasq = cpool.tile([128, H], F32, tag="bq")
            biask = cpool.tile([128, H], F32, tag="bk")
            biasv = cpool.tile([128, H], F32, tag="bvt")
            bor = cpool.tile([1, D], F16, tag="bo")
            onesr = cpool.tile([1, 128], F16, tag="ones")
            nc.sync.dma_start(biasq[:], bq2[:])
            nc.sync.dma_start(biask[:], bk2[:])
            nc.sync.dma_start(biasv[:], bv2[:])
            nc.sync.dma_start(bor[:], bo_row[:])
            nc.sync.dma_start(onesr[:], ones_row[:])
            shiftc = cpool.tile([128, 1], F32, tag="shiftc")
            nc.vector.memset(shiftc[:], -SHIFT)

            def transpose_weight(dst, wg, wnp, psp, tag, pstag):
                # wg DRAM [j, d] natural -> dst SBUF [128 d-part, dt, j]
                for jt in range(KT):
                    wn = wnp.tile([128, D], F16, tag=tag)
                    nc.sync.dma_start(wn[:], wg[jt * 128:(jt + 1) * 128, :])
                    for dt in range(KT):
                        pw = psp.tile([128, 128], F16, tag=pstag)
                        nc.tensor.transpose(pw[:], wn[:, dt * 128:(dt + 1) * 128], identF[:])
                        nc.any.tensor_copy(dst[:, dt, jt * 128:(jt + 1) * 128], pw[:])

            # ---------------- Phase A: QKV projections ----------------
            for xin, wg, bias, spills in (
                (xq, Wg[0], biasq, QT_ds),
                (xk, Wg[1], biask, KT_ds),
                (xv, Wg[2], biasv, VT_ds),
            ):
                with (
                    tc.tile_pool(name="wt", bufs=1) as wtp,
                    tc.tile_pool(name="wnp", bufs=2) as wnp,
                    tc.tile_pool(name="xb", bufs=2) as xbp,
                    tc.tile_pool(name="xt", bufs=2) as xtp,
                    tc.tile_pool(name="stA", bufs=2) as stp,
                    tc.tile_pool(name="psA", bufs=4, space="PSUM") as psA,
                    tc.tile_pool(name="psT", bufs=4, space="PSUM") as psT,
                ):
                    WT = wtp.tile([128, KT, D], F16, tag="WT", name="WT")
                    transpose_weight(WT, wg, wnp, psT, "wn", "psT")
                    for c in range(NCH):
                        xn = xbp.tile([128, NTB, D], F16, tag="xn")
                        nc.sync.dma_start(
                            xn[:], xin[c * TA:(c + 1) * TA, :].rearrange(
                                "(tb p) d -> p tb d", p=128))
                        xT = xtp.tile([128, KT, TA], F16, tag="xT")
                        for tb in range(NTB):
                            for kk in range(KT):
                                px = psT.tile([128, 128], F16, tag="psT")
                                nc.tensor.transpose(
                                    px[:], xn[:, tb, kk * 128:(kk + 1) * 128], identF[:])
                                nc.any.tensor_copy(
                                    xT[:, kk, tb * 128:(tb + 1) * 128], px[:])
                        stg = stp.tile([128, TA, H], F16, tag="stA")
                        for jg in range(4):
                            pss = [psA.tile([128, TA], F32, tag="psA",
                                            name=f"psA{jg}_{j}") for j in range(4)]
                            for kk in range(KT):
                                for jl in range(4):
                                    jt = jg * 4 + jl
                                    nc.tensor.matmul(
                                        pss[jl][:], WT[:, kk, jt * 128:(jt + 1) * 128],
                                        xT[:, kk, :], start=(kk == 0), stop=(kk == KT - 1))
                            for jl in range(4):
                                jt = jg * 4 + jl
                                nc.any.tensor_scalar_add(stg[:, :, jt], pss[jl][:],
                                                         bias[:, jt:jt + 1])
                        nc.sync.dma_start(spills[c][:], stg[:].rearrange("p t h -> p (t h)"))

            # ------- Phase B+C fused: per-token attention + out proj -------
            with (
                tc.tile_pool(name="wo", bufs=1) as wop,
                tc.tile_pool(name="wnp2", bufs=2) as wnp2,
                tc.tile_pool(name="qk", bufs=2) as qkp,
                tc.tile_pool(name="vbp", bufs=2) as vbp,
                tc.tile_pool(name="attc", bufs=2) as atp,
                tc.tile_pool(name="eb", bufs=6) as ebp,
                tc.tile_pool(name="zb", bufs=8) as zbp,
                tc.tile_pool(name="stC", bufs=4) as stp2,
                tc.tile_pool(name="psS", bufs=2, space="PSUM") as psS,
                tc.tile_pool(name="psT2", bufs=2, space="PSUM") as psT2,
                tc.tile_pool(name="psA2", bufs=2, space="PSUM") as psA2,
                tc.tile_pool(name="psC", bufs=2, space="PSUM") as psC,
            ):
                WoT = wop.tile([128, KT, D], F16, tag="WoT", name="WoT")
                transpose_weight(WoT, Wg[3], wnp2, psT2, "wn2", "ps16")
                for c in range(NCH):
                    QTs = qkp.tile([128, TA, H], F16, tag="QTs")
                    KTs = qkp.tile([128, TA, H], F16, tag="KTs")
                    VTs = vbp.tile([128, TA, H], F16, tag="VTs")
                    nc.sync.dma_start(QTs[:], QT_ds[c][:].rearrange("p (t h) -> p t h", h=H))
                    nc.sync.dma_start(KTs[:], KT_ds[c][:].rearrange("p (t h) -> p t h", h=H))
                    nc.sync.dma_start(VTs[:], VT_ds[c][:].rearrange("p (t h) -> p t h", h=H))
                    ATTc = atp.tile([128, H, TA], F16, tag="ATTc")
                    for bk in range(NBK):
                        sl = slice(bk * 8, (bk + 1) * 8)
                        ps_b = psS.tile([128, 128], F32, tag="psS")
                        nc.tensor.matmul(
                            ps_b[:],
                            QTs[:, sl, :].rearrange("p t h -> p (t h)"),
                            KTs[:, sl, :].rearrange("p t h -> p (t h)"),
                            start=True, stop=False, skip_group_check=True)
                        nc.tensor.matmul(ps_b[:], u8[:], v8[:],
                                         start=False, stop=True, skip_group_check=True)
                        E = ebp.tile([128, 128], F32, tag="E")
                        Z = zbp.tile([128, 1], F32, tag="Z")
                        nc.scalar.activation(E[:], ps_b[:], Exp,
                                             bias=shiftc[:], accum_out=Z[:])
                        R = zbp.tile([128, 1], F32, tag="R")
                        nc.vector.reciprocal(R[:], Z[:])
                        Wb = ebp.tile([128, 128], F16, tag="Wb")
                        nc.vector.tensor_scalar_mul(Wb[:], E[:], R[:])
                        ps_t = psT2.tile([128, 128], F16, tag="ps16")
                        nc.tensor.transpose(ps_t[:], Wb[:], identF[:])
                        WTs = ebp.tile([128, 128], F16, tag="WTs")
                        nc.any.tensor_copy(WTs[:], ps_t[:])
                        ps_v = psT2.tile([128, 128], F16, tag="ps16")
                        nc.tensor.transpose(
                            ps_v[:], VTs[:, sl, :].rearrange("p t h -> p (t h)"), identF[:])
                        Vb = ebp.tile([128, 128], F16, tag="Vb")
                        nc.any.tensor_copy(Vb[:], ps_v[:])
                        ps_a = psA2.tile([128, 128], F32, tag="psA2")
                        nc.tensor.matmul(ps_a[:], Vb[:], WTs[:], start=True, stop=True)
                        nc.any.tensor_copy(
                            ATTc[:, :, bk * 8:(bk + 1) * 8].rearrange("p h t -> p t h"),
                            ps_a[:].rearrange("p (t h) -> p t h", t=8))
                    # output projection for this chunk (attn stays in SBUF)
                    for tb in range(NTB):
                        for jc in range(D // 512):
                            ps = psC.tile([128, 512], F32, tag="psC")
                            for hh in range(KT):
                                nc.tensor.matmul(
                                    ps[:], ATTc[:, hh, tb * 128:(tb + 1) * 128],
                                    WoT[:, hh, jc * 512:(jc + 1) * 512],
                                    start=(hh == 0), stop=False)
                            nc.tensor.matmul(ps[:], onesr[:], bor[:, jc * 512:(jc + 1) * 512],
                                             start=False, stop=True)
                            st = stp2.tile([128, 512], F16, tag="stC")
                            nc.any.tensor_copy(st[:], ps[:])
                            nc.sync.dma_start(
                                out_d[c * TA + tb * 128:c * TA + (tb + 1) * 128,
                                      jc * 512:(jc + 1) * 512], st[:])
    nc.compile()
    return nc


_cache = {}


def get_nc(T):
    if T not in _cache:
        _cache[T] = build(T)
    return _cache[T]


def make_in_maps(q, k, v, Wq, bq, Wk, bk, Wv, bv, Wo, bo, ncores=NCORES, T=None):
    f16, f32 = np.float16, np.float32
    q = np.asarray(q, f32).reshape(-1, D).astype(f16)
    k = np.asarray(k, f32).reshape(-1, D).astype(f16)
    v = np.asarray(v, f32).reshape(-1, D).astype(f16)
    if T is None:
        T = q.shape[0] // ncores
    DS_ = D // ncores
    W16 = [np.asarray(W, f32).astype(f16) for W in (Wq, Wk, Wv, Wo)]
    b2 = [np.ascontiguousarray(np.asarray(b, f32).reshape(H, 128).T)
          for b in (bq, bk, bv)]
    bo_row = np.asarray(bo, f32).astype(f16).reshape(1, D)
    ones = np.ones((1, 128), f16)
    maps = []
    for c in range(ncores):
        sl = slice(c * T, (c + 1) * T)
        ws = slice(c * DS_, (c + 1) * DS_)
        maps.append({
            "xq": q[sl], "xk": k[sl], "xv": v[sl],
            "wqs": W16[0][ws], "wks": W16[1][ws],
            "wvs": W16[2][ws], "wos": W16[3][ws],
            "bq2": b2[0], "bk2": b2[1], "bv2": b2[2],
            "bo_row": bo_row, "ones_row": ones,
        })
    return maps, T


def kernel(q, k, v, Wq, bq, Wk, bk, Wv, bv, Wo, bo):
    maps, T = make_in_maps(q, k, v, Wq, bq, Wk, bk, Wv, bv, Wo, bo)
    nc = get_nc(T)
    res = run_bass_kernel_spmd(nc, maps, list(range(NCORES)))
    out = np.concatenate([np.asarray(r["out"]) for r in res.results], axis=0)
    return out.reshape(B, S, D).astype(np.float32)


# revision 22
# speedup vs baseline: 2.9399x; 1.0378x over previous
"""Trainium2 Bass kernel for nn_MultiHeadAttention_67379446939752.

Per-token multi-head attention:
  Q = q @ Wq.T + bq ; K,V likewise        [B,S,D] -> [B,S,H,HD]
  score[t,h,g] = sum_d Q[t,h,d] K[t,g,d]  (per-token HxH gram, no seq mixing)
  attn[t] = softmax(score[t]) @ V[t]      -> [B,S,D]
  out = attn @ Wo.T + bo

v2 strategy (wall-clock per call is dominated by host<->device transfer, so
minimize wire bytes first, then keep HW exec near the PE roofline):
  - Data-parallel over the 16384 tokens across 8 NeuronCores (2048/core).
  - fp16 wire format for activations, weights and output (max rel err vs
    fp64 reference ~2.4e-3, an 8x margin under the 2e-2 gate).
  - Weights are sharded 8-ways on the wire (256 rows each) and AllGathered
    on-device over NeuronLink: 536MB of replicated weight traffic -> 33.5MB.
  - Natural [T,D]/[D,D] row-major layouts on the wire; the contraction-dim
    transposes happen on-device as cheap PE transpose ops (fp16: 128 cyc per
    128x128 tile), so the host does no big transposes.
  - All big matmuls in fp16 operands (full PE rate, fp32 PSUM accumulate).
  - The per-token 16x16 attention runs 8 tokens at a time as 128x128 fp16
    matmuls whose cross-token blocks are pushed to -1024 in PSUM by a rank-8
    mask matmul; exp() then zeroes them exactly (block-diagonal softmax with
    no DVE masking pass).
  - Attention and the output projection are fused per 256-token chunk (the
    attn result never round-trips through DRAM).
"""
import sys
sys.path.insert(0, "/opt/trn_rl_repo")
import numpy as np
import concourse.bass as bass
import concourse.mybir as mybir
import concourse.bacc as bacc
import concourse.tile as tile
from concourse.bass_utils import run_bass_kernel_spmd

B, S, D, H, HD = 4, 4096, 2048, 16, 128
NCORES = 8
T_FULL = B * S
F16, F32 = mybir.dt.float16, mybir.dt.float32
KT = D // 128            # contraction tiles
SHIFT = 25.0             # constant softmax shift (softmax-invariant)
NEG = 1024.0             # additive mask magnitude for cross-token blocks
TA = 256                 # token chunk
Exp = mybir.ActivationFunctionType.Exp


def mask_consts():
    # u8[r,(t,h)] = 1 if t==r ; v8[r,(t',g)] = -NEG*(1 - (t'==r))
    u = np.zeros((8, 128), np.float16)
    for r in range(8):
        u[r, r * 16:(r + 1) * 16] = 1.0
    v = np.full((8, 128), -NEG, np.float16)
    for r in range(8):
        v[r, r * 16:(r + 1) * 16] = 0.0
    return u, v


def build(T, ncores=NCORES, shared_gather=True):
    NCH = T // TA            # chunks
    NBK = TA // 8            # 8-token blocks per chunk
    NTB = TA // 128          # 128-token row tiles per chunk
    DS_ = D // ncores        # weight shard rows
    nc = bacc.Bacc(None, target_bir_lowering=False, num_devices=ncores)
    xq = nc.dram_tensor("xq", [T, D], F16, kind="ExternalInput")
    xk = nc.dram_tensor("xk", [T, D], F16, kind="ExternalInput")
    xv = nc.dram_tensor("xv", [T, D], F16, kind="ExternalInput")
    wqs = nc.dram_tensor("wqs", [DS_, D], F16, kind="ExternalInput")
    wks = nc.dram_tensor("wks", [DS_, D], F16, kind="ExternalInput")
    wvs = nc.dram_tensor("wvs", [DS_, D], F16, kind="ExternalInput")
    wos = nc.dram_tensor("wos", [DS_, D], F16, kind="ExternalInput")
    bq2 = nc.dram_tensor("bq2", [128, H], F32, kind="ExternalInput")
    bk2 = nc.dram_tensor("bk2", [128, H], F32, kind="ExternalInput")
    bv2 = nc.dram_tensor("bv2", [128, H], F32, kind="ExternalInput")
    bo_row = nc.dram_tensor("bo_row", [1, D], F16, kind="ExternalInput")
    ones_row = nc.dram_tensor("ones_row", [1, 128], F16, kind="ExternalInput")
    out_d = nc.dram_tensor("out", [T, D], F16, kind="ExternalOutput")

    u8_np, v8_np = mask_consts()
    u8_d = nc.inline_tensor(u8_np, "u8c")
    v8_d = nc.inline_tensor(v8_np, "v8c")
    id_d = nc.inline_tensor(np.eye(128, dtype=np.float16), "id128")

    with tile.TileContext(nc) as tc:
        with (
            tc.tile_pool(name="dram", bufs=1, space="DRAM") as dpool,
            tc.tile_pool(name="const", bufs=1) as cpool,
        ):
            u8 = cpool.tile([8, 128], F16, tag="u8")
            v8 = cpool.tile([8, 128], F16, tag="v8")
            identF = cpool.tile([128, 128], F16, tag="identF")
            nc.sync.dma_start(u8[:], u8_d[:])
            nc.sync.dma_start(v8[:], v8_d[:])
            nc.sync.dma_start(identF[:], id_d[:])
            biasq = cpool.tile([128, H], F32, tag="bq")
            biask = cpool.tile([128, H], F32, tag="bk")
            biasv = cpool.tile([128, H], F32, tag="bvt")
            bor = cpool.tile([1, D], F16, tag="bo")
            onesr = cpool.tile([1, 128], F16, tag="ones")
            nc.sync.dma_start(biasq[:], bq2[:])
            nc.sync.dma_start(biask[:], bk2[:])
            nc.sync.dma_start(biasv[:], bv2[:])
            nc.sync.dma_start(bor[:], bo_row[:])
            nc.sync.dma_start(onesr[:], ones_row[:])
            shiftc = cpool.tile([128, 1], F32, tag="shiftc")
            nc.vector.memset(shiftc[:], -SHIFT)

            # ---- weight shards: transpose locally (no gather dep), then
            # AllGather pre-transposed shards straight into W^T layout ----
            # WgT[i] is [D(d), D(j)] fp16 = W^T; rank c's contribution lands in
            # columns [c*DS_, (c+1)*DS_) via the rank-major output AP.
            WgT = []
            NSB = DS_ // 128         # 128-row blocks per shard
            with (
                tc.tile_pool(name="shx", bufs=2) as shp,
                tc.tile_pool(name="sht", bufs=2) as stp0,
                tc.tile_pool(name="psSh", bufs=4, space="PSUM") as psh,
            ):
                for i, wsh in enumerate((wqs, wks, wvs, wos)):
                    # rank-major contiguous gather output: block c is
                    # W^T[:, c*DS_:(c+1)*DS_] as a [D, DS_] tile
                    gg = dpool.tile([ncores * D, DS_], F16, tag=f"WgT{i}",
                                    name=f"WgT{i}",
                                    addr_space="Shared" if shared_gather else "Local")
                    wtb = dpool.tile([D, DS_], F16, tag=f"wtb{i}", name=f"wtb{i}")
                    sh = shp.tile([128, NSB, D], F16, tag="sh")
                    nc.sync.dma_start(
                        sh[:], wsh[:].rearrange("(b p) d -> p b d", p=128))
                    shT = stp0.tile([128, KT, DS_], F16, tag="shT")
                    for b in range(NSB):
                        for dh in range(KT // 8):
                            pw = psh.tile([128, 8, 128], F16, tag="psSh")
                            for dl in range(8):
                                dt = dh * 8 + dl
                                nc.tensor.matmul(
                                    pw[:, dl, :], sh[:, b, dt * 128:(dt + 1) * 128],
                                    identF[:], is_transpose=True, skip_group_check=True)
                            nc.any.tensor_copy(
                                shT[:, dh * 8:(dh + 1) * 8, b * 128:(b + 1) * 128],
                                pw[:])
                    nc.sync.dma_start(
                        wtb[:].rearrange("(dt p) jl -> p dt jl", p=128), shT[:])
                    if ncores == 1:
                        nc.gpsimd.dma_start(gg[:], wtb[:])
                    else:
                        nc.gpsimd.collective_compute(
                            "AllGather", mybir.AluOpType.bypass,
                            replica_groups=[list(range(ncores))],
                            ins=[wtb[:]], outs=[gg[:]])
                    WgT.append(gg)

            # per-chunk QKV spill tiles (fine-grained cross-phase deps)
            QT_ds = [dpool.tile([128, TA * H], F16, tag=f"QTd{i}", name=f"QTd{i}") for i in range(NCH)]
            KT_ds = [dpool.tile([128, TA * H], F16, tag=f"KTd{i}", name=f"KTd{i}") for i in range(NCH)]
            VT_ds = [dpool.tile([128, TA * H], F16, tag=f"VTd{i}", name=f"VTd{i}") for i in range(NCH)]

            NBC = 512 // DS_         # rank blocks per 512-col quarter

            def load_wt_quarters(pool, wg, tag):
                # gathered W^T DRAM [(c d), jl] rank-major -> 4 SBUF tiles
                # [128 d-part, dt, 512 j]; quarter q covers rank blocks
                # c in [q*NBC, (q+1)*NBC)
                parts = []
                for q in range(4):
                    wq_ = pool.tile([128, KT, 512], F16, tag=f"{tag}{q}",
                                    name=f"{tag}{q}")
                    for b in range(NBC):
                        c = q * NBC + b
                        nc.sync.dma_start(
                            wq_[:, :, b * DS_:(b + 1) * DS_],
                            wg[c * D:(c + 1) * D, :].rearrange(
                                "(dt p) jl -> p dt jl", p=128))
                    parts.append(wq_)
                return parts

            # ---------------- Phase A: QKV projections ----------------
            for xin, wg, bias, spills in (
                (xq, WgT[0], biasq, QT_ds),
                (xk, WgT[1], biask, KT_ds),
                (xv, WgT[2], biasv, VT_ds),
            ):
                with (
                    tc.tile_pool(name="wt", bufs=1) as wtp,
                    tc.tile_pool(name="xb", bufs=2) as xbp,
                    tc.tile_pool(name="xt", bufs=2) as xtp,
                    tc.tile_pool(name="stA", bufs=2) as stp,
                    tc.tile_pool(name="psA", bufs=4, space="PSUM") as psA,
                    tc.tile_pool(name="psT", bufs=4, space="PSUM") as psT,
                ):
                    WT = load_wt_quarters(wtp, wg, "WT")
                    for c in range(NCH):
                        xn = xbp.tile([128, NTB, D], F16, tag="xn")
                        nc.sync.dma_start(
                            xn[:], xin[c * TA:(c + 1) * TA, :].rearrange(
                                "(tb p) d -> p tb d", p=128))
                        xT = xtp.tile([128, KT, TA], F16, tag="xT")
                        for tb in range(NTB):
                            for kh in range(KT // 8):
                                px = psT.tile([128, 8, 128], F16, tag="psT")
                                for kl in range(8):
                                    kk = kh * 8 + kl
                                    nc.tensor.matmul(
                                        px[:, kl, :], xn[:, tb, kk * 128:(kk + 1) * 128],
                                        identF[:], is_transpose=True, skip_group_check=True)
                                nc.any.tensor_copy(
                                    xT[:, kh * 8:(kh + 1) * 8, tb * 128:(tb + 1) * 128],
                                    px[:])
                        stg = stp.tile([128, TA, H], F16, tag="stA")
                        for jg in range(4):
                            pss = [psA.tile([128, TA], F32, tag="psA",
                                            name=f"psA{jg}_{j}") for j in range(4)]
                            for kk in range(KT):
                                for jl in range(4):
                                    nc.tensor.matmul(
                                        pss[jl][:],
                                        WT[jg][:, kk, jl * 128:(jl + 1) * 128],
                                        xT[:, kk, :], start=(kk == 0), stop=(kk == KT - 1))
                            for jl in range(4):
                                jt = jg * 4 + jl
                                nc.any.tensor_scalar_add(stg[:, :, jt], pss[jl][:],
                                                         bias[:, jt:jt + 1])
                        nc.sync.dma_start(spills[c][:], stg[:].rearrange("p t h -> p (t h)"))

            # ------- Phase B+C fused: per-token attention + out proj -------
            with (
                tc.tile_pool(name="wo", bufs=1) as wop,
                tc.tile_pool(name="qk", bufs=2) as qkp,
                tc.tile_pool(name="vbp", bufs=2) as vbp,
                tc.tile_pool(name="attc", bufs=2) as atp,
                tc.tile_pool(name="eb", bufs=6) as ebp,
                tc.tile_pool(name="zb", bufs=8) as zbp,
                tc.tile_pool(name="stC", bufs=4) as stp2,
                tc.tile_pool(name="psS", bufs=2, space="PSUM") as psS,
                tc.tile_pool(name="psT2", bufs=2, space="PSUM") as psT2,
                tc.tile_pool(name="psA2", bufs=2, space="PSUM") as psA2,
                tc.tile_pool(name="psC", bufs=2, space="PSUM") as psC,
            ):
                WoT = load_wt_quarters(wop, WgT[3], "WoT")
                NG = NBK // 4           # groups of 4 blocks (32 tokens)
                for c in range(NCH):
                    QTs = qkp.tile([128, TA, H], F16, tag="QTs")
                    KTs = qkp.tile([128, TA, H], F16, tag="KTs")
                    VTs = vbp.tile([128, TA, H], F16, tag="VTs")
                    nc.gpsimd.dma_start(QTs[:], QT_ds[c][:].rearrange("p (t h) -> p t h", h=H))
                    nc.gpsimd.dma_start(KTs[:], KT_ds[c][:].rearrange("p (t h) -> p t h", h=H))
                    nc.gpsimd.dma_start(VTs[:], VT_ds[c][:].rearrange("p (t h) -> p t h", h=H))
                    ATTc = atp.tile([128, H, TA], F16, tag="ATTc")

                    def issue_scores(g):
                        # scores for 4 blocks -> one packed PSUM bank
                        psb = psS.tile([128, 4, 128], F32, tag="psS", name=f"psb{c}_{g}")
                        for i in range(4):
                            sl = slice((g * 4 + i) * 8, (g * 4 + i + 1) * 8)
                            nc.tensor.matmul(
                                psb[:, i, :],
                                QTs[:, sl, :].rearrange("p t h -> p (t h)"),
                                KTs[:, sl, :].rearrange("p t h -> p (t h)"),
                                start=True, stop=False, skip_group_check=True)
                            nc.tensor.matmul(psb[:, i, :], u8[:], v8[:],
                                             start=False, stop=True, skip_group_check=True)
                        return psb

                    def issue_attend(g, psb):
                        # softmax (ACT/DVE) then transposes + attn matmuls (PE)
                        E = ebp.tile([128, 4, 128], F32, tag="E")
                        Z4 = zbp.tile([128, 4], F32, tag="Z4")
                        for i in range(4):
                            nc.scalar.activation(E[:, i, :], psb[:, i, :], Exp,
                                                 bias=shiftc[:], accum_out=Z4[:, i:i + 1])
                        R4 = zbp.tile([128, 4], F32, tag="R4")
                        nc.vector.reciprocal(R4[:], Z4[:])
                        Wb = ebp.tile([128, 4, 128], F16, tag="Wb")
                        for i in range(4):
                            nc.vector.tensor_scalar_mul(Wb[:, i, :], E[:, i, :],
                                                        R4[:, i:i + 1])
                        pt = psT2.tile([128, 8, 128], F16, tag="ps16")
                        for i in range(4):
                            sl = slice((g * 4 + i) * 8, (g * 4 + i + 1) * 8)
                            nc.tensor.matmul(pt[:, i, :], Wb[:, i, :], identF[:],
                                             is_transpose=True, skip_group_check=True)
                            nc.tensor.matmul(
                                pt[:, 4 + i, :],
                                VTs[:, sl, :].rearrange("p t h -> p (t h)"), identF[:],
                                is_transpose=True, skip_group_check=True)
                        WVb = ebp.tile([128, 8, 128], F16, tag="WVb")
                        nc.any.tensor_copy(WVb[:], pt[:])
                        psa = psA2.tile([128, 4, 128], F32, tag="psA2")
                        for i in range(4):
                            nc.tensor.matmul(psa[:, i, :], WVb[:, 4 + i, :],
                                             WVb[:, i, :], start=True, stop=True,
                                             skip_group_check=True)
                        nc.any.tensor_copy(
                            ATTc[:, :, g * 32:(g + 1) * 32].rearrange(
                                "p h (b t) -> p b t h", b=4),
                            psa[:].rearrange("p b (t h) -> p b t h", t=8))

                    prev = issue_scores(0)
                    for g in range(1, NG):
                        cur = issue_scores(g)
                        issue_attend(g - 1, prev)
                        prev = cur
                    issue_attend(NG - 1, prev)
                    # output projection for this chunk (attn stays in SBUF)
                    for tb in range(NTB):
                        for jc in range(D // 512):
                            ps = psC.tile([128, 512], F32, tag="psC")
                            for hh in range(KT):
                                nc.tensor.matmul(
                                    ps[:], ATTc[:, hh, tb * 128:(tb + 1) * 128],
                                    WoT[jc][:, hh, :],
                                    start=(hh == 0), stop=False)
                            nc.tensor.matmul(ps[:], onesr[:], bor[:, jc * 512:(jc + 1) * 512],
                                             start=False, stop=True)
                            st = stp2.tile([128, 512], F16, tag="stC")
                            nc.any.tensor_copy(st[:], ps[:])
                            nc.sync.dma_start(
                                out_d[c * TA + tb * 128:c * TA + (tb + 1) * 128,
                                      jc * 512:(jc + 1) * 512], st[:])
    nc.compile()
    return nc


_cache = {}


def get_nc(T):
    if T not in _cache:
        _cache[T] = build(T)
    return _cache[T]


def make_in_maps(q, k, v, Wq, bq, Wk, bk, Wv, bv, Wo, bo, ncores=NCORES, T=None):
    f16, f32 = np.float16, np.float32
    q = np.asarray(q, f32).reshape(-1, D).astype(f16)
    k = np.asarray(k, f32).reshape(-1, D).astype(f16)
    v = np.asarray(v, f32).reshape(-1, D).astype(f16)
    if T is None:
        T = q.shape[0] // ncores
    DS_ = D // ncores
    W16 = [np.asarray(W, f32).astype(f16) for W in (Wq, Wk, Wv, Wo)]
    b2 = [np.ascontiguousarray(np.asarray(b, f32).reshape(H, 128).T)
          for b in (bq, bk, bv)]
    bo_row = np.asarray(bo, f32).astype(f16).reshape(1, D)
    ones = np.ones((1, 128), f16)
    maps = []
    for c in range(ncores):
        sl = slice(c * T, (c + 1) * T)
        ws = slice(c * DS_, (c + 1) * DS_)
        maps.append({
            "xq": q[sl], "xk": k[sl], "xv": v[sl],
            "wqs": W16[0][ws], "wks": W16[1][ws],
            "wvs": W16[2][ws], "wos": W16[3][ws],
            "bq2": b2[0], "bk2": b2[1], "bv2": b2[2],
            "bo_row": bo_row, "ones_row": ones,
        })
    return maps, T


def kernel(q, k, v, Wq, bq, Wk, bk, Wv, bv, Wo, bo):
    maps, T = make_in_maps(q, k, v, Wq, bq, Wk, bk, Wv, bv, Wo, bo)
    nc = get_nc(T)
    res = run_bass_kernel_spmd(nc, maps, list(range(NCORES)))
    out = np.concatenate([np.asarray(r["out"]) for r in res.results], axis=0)
    return out.reshape(B, S, D).astype(np.float32)
